# revision 9
# baseline (speedup 1.0000x reference)
"""Inverse 2D Haar wavelet (conv_transpose2d, kernel=stride=2, groups=C) on 8 trn2 cores.

Input  x  [B, 4C, H, W]  (B=16, C=64, H=W=128), subbands a,b,c,d per channel.
Output y  [B, C, 2H, 2W] with, per pixel (h, w):
    y[2h+0, 2w+0] = a - b - c + d      = (a-b) - (c-d) = u - s
    y[2h+0, 2w+1] = a - b + c - d      = (a-b) + (c-d) = u + s
    y[2h+1, 2w+0] = a + b - c - d      = (a+b) - (c+d) = v - t
    y[2h+1, 2w+1] = a + b + c + d      = (a+b) + (c+d) = v + t

Sharding: pure data-parallel over batch, 2 images per core.

Per-core layout: SBUF partition p = (image, channel) — 2*64 = 128 — and the
free dim holds (subband k, row-block, w) for a horizontal stripe of hb image
rows. The (image, channel) dims merge into a single stride-contiguous DMA dim,
so each iteration is ONE big load (contiguous 4*hb*W runs per subband) and ONE
big store (2hb*2W contiguous output rows per partition).

The butterfly is split across two engines with no cross-engine data deps:
    DVE    : s = c-d, u = a-b, then y(2h,2w)=u-s, y(2h,2w+1)=u+s
    GpSimd : t = c+d, v = a+b, then y(2h+1,2w)=v-t, y(2h+1,2w+1)=v+t
Each engine writes a disjoint half of the out tile (even/odd output rows).
With the split, elementwise work is fully hidden behind DMA: the measured
per-execution device time equals a DMA-only (load+store, no compute) probe
(~230us/core for 64MB of HBM traffic, ~78% of the 358GB/s per-core peak).

(h, p) output interleave falls out of the free-dim row layout; (w, q) is fused
into the stage-2 butterfly ops as strided SBUF writes.
"""

import numpy as np

B, C, H, W = 16, 64, 128, 128
N_CORES = 8
B_PER_CORE = B // N_CORES

_PROGRAM_CACHE = {}

# Haar subband weights this kernel hardcodes (k, p, q) — must match `filters`.
_HAAR = np.array(
    [
        [[1.0, 1.0], [1.0, 1.0]],     # ll
        [[-1.0, -1.0], [1.0, 1.0]],   # lh
        [[-1.0, 1.0], [-1.0, 1.0]],   # hl
        [[1.0, -1.0], [-1.0, 1.0]],   # hh
    ],
    dtype=np.float32,
)


def build_program(b2=B_PER_CORE, c=C, h=H, w=W, hb=8, bufs=5, reps=1, drain=False):
    """Per-core Bass program (raw bass, hand-rolled sync: the installed walrus
    rejects instructions with more than one sync-wait, which rules out Tile's
    conservative non-transitive dep tracking).

    4-queue pipeline over n_it = h/hb stripe iterations:
      SP     : load stripe          (one 128-partition DMA, 4*hb*w floats/part)
      DVE    : 4 butterfly TT ops   (even output rows, w/q-interleaved writes)
      GPSIMD : 4 butterfly TT ops   (odd output rows)
      ACT    : store stripe         (one DMA, 2hb contiguous output rows/part)

    `reps` re-runs the whole transform back-to-back inside one program
    (same output, reps x the HBM traffic) — a timing probe used by test.py to
    amplify device-side execution above the axon dispatch floor; the graded
    kernel uses reps=1. With `drain=True` the first load of each rep waits for
    every store of the previous rep, so reps do not share pipeline fill/drain:
    the per-rep marginal then measures a COLD single execution, not sustained
    throughput.
    """
    import concourse.bass as bass
    import concourse.mybir as mybir
    from contextlib import ExitStack

    p_n = b2 * c                 # SBUF partitions used (= 128 at full scale)
    assert p_n <= 128 and h % hb == 0
    n_it = h // hb
    fd = hb * w                  # free-dim elements per subband per partition

    dt = mybir.dt.float32
    nc = bass.Bass("TRN2", target_bir_lowering=False, debug=False)
    x = nc.dram_tensor("x", [b2, 4 * c, h, w], dt, kind="ExternalInput").ap()
    y = nc.dram_tensor("y", [b2, c, 2 * h, 2 * w], dt, kind="ExternalOutput").ap()

    # [ (bb c), k, h, w ] — (bb c) merges to one DMA dim (stride-contiguous).
    xv = x.rearrange("bb (c k) h w -> (bb c) k h w", k=4)
    # [ (bb c), (h2 w2) ] — per-partition flat output plane.
    yv = y.rearrange("bb c h2 w2 -> (bb c) (h2 w2)")

    in_tiles = [
        nc.alloc_sbuf_tensor(f"tin{j}", [p_n, 4 * fd], dt).ap() for j in range(bufs)
    ]
    # tmp planes: 0=s, 1=u (DVE); 2=t, 3=v (GPSIMD). One shared (unbuffered)
    # tile: each engine produces and consumes its planes within the same
    # iteration and is self-serialized, so no slot rotation is needed — the
    # freed SBUF goes to deeper in/out buffering instead.
    tmp = nc.alloc_sbuf_tensor("ttmp", [p_n, 4 * fd], dt).ap()
    out_tiles = [
        nc.alloc_sbuf_tensor(f"tout{j}", [p_n, 4 * fd], dt).ap() for j in range(bufs)
    ]

    stripes = list(range(n_it)) * reps
    N = len(stripes)

    with ExitStack() as ctx:
        # Per-slot DMA sems: a single sem shared by two in-flight DMAs is racy
        # (each DMA is 16 independent +1s; a mixed 16 wouldn't mean DMA 0 done).
        load_sems = [
            ctx.enter_context(nc.semaphore(f"load_sem{j}")) for j in range(bufs)
        ]
        store_sems = [
            ctx.enter_context(nc.semaphore(f"store_sem{j}")) for j in range(bufs)
        ]
        dve_sem = ctx.enter_context(nc.semaphore("dve_sem"))
        gps_sem = ctx.enter_context(nc.semaphore("gps_sem"))
        block = ctx.enter_context(nc.Block())

        @block.sync
        def _(sync):
            for it in range(N):
                st = stripes[it]
                if drain and it > 0 and it % n_it == 0:
                    # rep barrier: all stores of the previous rep complete
                    for j in range(bufs):
                        n_st = sum(1 for k in range(it) if k % bufs == j)
                        sync.wait_ge(store_sems[j], 16 * n_st)
                if it >= bufs:
                    # WAR: slot's previous stripe fully consumed by both
                    # compute engines (transitively orders vs the slot's
                    # previous load too).
                    sync.wait_ge(dve_sem, 4 * (it - bufs + 1))
                    sync.wait_ge(gps_sem, 4 * (it - bufs + 1))
                sync.dma_start(
                    out=in_tiles[it % bufs].rearrange(
                        "p (k hr w) -> p k hr w", k=4, hr=hb
                    ),
                    in_=xv[:, :, st * hb : (st + 1) * hb, :],
                ).then_inc(load_sems[it % bufs], 16)

        @block.scalar
        def _(scalar):
            for it in range(N):
                st = stripes[it]
                scalar.wait_ge(dve_sem, 4 * (it + 1))
                scalar.wait_ge(gps_sem, 4 * (it + 1))
                scalar.dma_start(
                    out=yv[:, st * 4 * fd : (st + 1) * 4 * fd],
                    in_=out_tiles[it % bufs],
                ).then_inc(store_sems[it % bufs], 16)

        def engine_prog(which):
            # which: 0 = DVE half (s, u, even rows), 1 = GPSIMD half (t, v, odd)
            def prog(eng):
                sem = dve_sem if which == 0 else gps_sem
                # Ops are self-serialized via sem (each op incs by 1, each
                # subsequent op waits the running count): the CoreSim race
                # model treats same-engine completion as async, and HW drains
                # the pipe between ops anyway, so this costs only the wait.
                n_ops = 0

                def tt(op, out, i0, i1):
                    nonlocal n_ops
                    if n_ops:
                        eng.wait_ge(sem, n_ops)
                    op(out, i0, i1).then_inc(sem, 1)
                    n_ops += 1

                for it in range(N):
                    t4 = in_tiles[it % bufs].rearrange(
                        "p (k hr w) -> p k hr w", k=4, hr=hb
                    )
                    a, b_, c_, d_ = (t4[:, k] for k in range(4))
                    tm = tmp.rearrange("p (k hr w) -> p k hr w", k=4, hr=hb)
                    eng.wait_ge(load_sems[it % bufs], 16 * (it // bufs + 1))
                    if which == 0:
                        s_, u_ = tm[:, 0], tm[:, 1]
                        tt(eng.tensor_sub, s_, c_, d_)      # s = c - d
                        tt(eng.tensor_sub, u_, a, b_)       # u = a - b
                    else:
                        t_, v_ = tm[:, 2], tm[:, 3]
                        tt(eng.tensor_add, t_, c_, d_)      # t = c + d
                        tt(eng.tensor_add, v_, a, b_)       # v = a + b

                    # free layout per partition: (hr, pp, w, q) == row-major
                    o5 = out_tiles[it % bufs].rearrange(
                        "p (hr pp w q) -> p hr pp w q", hr=hb, pp=2, w=w, q=2
                    )
                    if it >= bufs:
                        # WAR: slot's previous stripe fully stored by ACT.
                        eng.wait_ge(store_sems[it % bufs], 16 * (it // bufs))
                    if which == 0:
                        tt(eng.tensor_sub, o5[:, :, 0, :, 0], u_, s_)  # y(2h,2w)
                        tt(eng.tensor_add, o5[:, :, 0, :, 1], u_, s_)  # y(2h,2w+1)
                    else:
                        tt(eng.tensor_sub, o5[:, :, 1, :, 0], v_, t_)  # y(2h+1,2w)
                        tt(eng.tensor_add, o5[:, :, 1, :, 1], v_, t_)  # y(2h+1,2w+1)
            return prog

        block.vector(engine_prog(0))
        block.gpsimd(engine_prog(1))
    return nc


def _get_program(reps=1, drain=False):
    key = (B_PER_CORE, C, H, W, reps, drain)
    if key not in _PROGRAM_CACHE:
        _PROGRAM_CACHE[key] = build_program(reps=reps, drain=drain)
    return _PROGRAM_CACHE[key]


def _reference_fallback(x, filters):
    # Generality net for non-Haar filters (not hit by the graded configuration).
    b, c4, h, w = x.shape
    c = c4 // 4
    f = filters.reshape(c, 4, 2, 2)
    xs = x.reshape(b, c, 4, h, w)
    yout = np.einsum("bckhw,ckpq->bchpwq", xs, f)
    return np.ascontiguousarray(yout.reshape(b, c, 2 * h, 2 * w))


def kernel(x, filters):
    x = np.asarray(x, dtype=np.float32)
    filters = np.asarray(filters, dtype=np.float32)

    f = filters.reshape(-1, 4, 2, 2)
    if not (f.shape[0] == C and np.array_equal(f, np.broadcast_to(_HAAR, f.shape))):
        return _reference_fallback(x, filters)

    from concourse.bass_utils import run_bass_kernel_spmd

    nc = _get_program()
    in_maps = [
        {"x": np.ascontiguousarray(x[i * B_PER_CORE : (i + 1) * B_PER_CORE])}
        for i in range(N_CORES)
    ]
    res = run_bass_kernel_spmd(nc, in_maps, list(range(N_CORES))).results
    return np.concatenate([res[i]["y"] for i in range(N_CORES)], axis=0)
